# revision 34
# baseline (speedup 1.0000x reference)
"""Tensor-parallel Llama attention (+LoRA) kernel for 8 trn2 NeuronCores.

Wall-clock in this environment is dominated by host->device transfer over the
axon tunnel (~35-55 MB/s), so the kernel is designed for minimum wire bytes:

- hidden_states + rope cos/sin ship fp16, sequence-sharded (1/8 per core,
  packed into one tensor), AllGathered on-device instead of replicated 8x.
- q/v/o weights ship int8 (native [out, in] layout, per-contraction-column
  scales); k stays fp16 (the scores path is the most error-sensitive).
  Transposition to lhsT layout and scale-dequant to fp16 happen on-device
  (PE transpose / XBAR DMA-transpose), so the host only casts/quantizes.
- The causal mask is generated on-device with affine_select (zero bytes);
  the softmax 1/sqrt(HD) scale is folded into the Exp activation.
- Output is fetched as fp16 and upcast on host.
Measured rel-err (fro) vs the fp32 reference: ~1.22e-2 of the 2e-2 budget.

Compute sharding (per spec hint): q heads column-wise (4 q-heads / core), kv
heads column-wise (1 kv-head / core, GQA aligned), o_w row-sharded on its
OUTPUT dim with an on-device AllGather of per-core attention outputs.
"""

import gc
import os

import numpy as np
import concourse.bass as bass
import concourse.mybir as mybir
from concourse import bacc
from concourse.tile import TileContext
from concourse.masks import make_identity
from concourse.bass_utils import run_bass_kernel_spmd

def _guard_trace_env():
    """The axon NTFF trace path needs antenv.axon_hooks; if BASS_TRACE is set
    but the module is missing, run_bass_kernel_spmd would crash on import.
    Fall back to untraced execution in that case. Called per kernel() call
    because callers may set BASS_TRACE after importing this module."""
    if os.environ.get("BASS_TRACE") and not os.environ.get("BASS_NEVER_TRACE"):
        try:
            import antenv.axon_hooks  # noqa: F401
        except Exception:
            os.environ["BASS_NEVER_TRACE"] = "1"

# run_bass_via_pjrt builds a fresh jax.jit per call; the persistent
# compilation cache turns the per-call XLA recompile into a disk hit.
try:
    import jax

    jax.config.update(
        "jax_compilation_cache_dir",
        os.environ.get("JAX_COMPILATION_CACHE_DIR", "/tmp/jax_comp_cache"))
    jax.config.update("jax_persistent_cache_min_compile_time_secs", 0.0)
    jax.config.update("jax_persistent_cache_min_entry_size_bytes", 0)
except Exception:
    pass

B, S, H = 1, 2048, 4096
NH, NKV, HD = 32, 8, 128
NCORES = 8
QH = NH // NCORES            # 4 q heads per core
EL = QH * HD                 # 512 local q/o columns
SSH = S // NCORES            # 256 sequence rows shipped per core
XA = H + 2 * HD              # x columns + cos + sin
ROPE_THETA = 10000.0
LORA_SCALE = 1.0
LR = 16                      # lora rank
KT = H // 128                # 32 contraction tiles
NSC = S // 512               # 4 sequence chunks of 512
NST = S // 128               # 16 k/s tiles of 128
SCALE = float(1.0 / np.sqrt(HD))
F32 = mybir.dt.float32
F16 = mybir.dt.float16
I8 = mybir.dt.int8
AF = mybir.ActivationFunctionType
ALU = mybir.AluOpType

LAST_RUN = None              # BassKernelResults of the most recent execution
_LAST_IN_MAPS = None
_PROGRAM_CACHE = {}


def _build_program(causal_ok: bool, with_lora: bool = True,
                   with_collective: bool = True):
    nc = bacc.Bacc(None, target_bir_lowering=False)

    x_sh = nc.declare_dram_parameter("x_sh", [SSH, XA], F16, isOutput=False)
    # q/v/o ship int8 in native [out, in] layout with per-contraction-row
    # fp32 scales packed as wsc[p, w*KT + kt] (w: q,v,o); the transpose to
    # lhsT layout and the scale-dequant to fp16 happen on-device.
    # k stays fp16 (scores are the most error-sensitive path) and ships in
    # native [out, in] layout; the XBAR DMA-transpose builds its lhsT tiles.
    wq8 = nc.declare_dram_parameter("wq8", [EL, H], I8, isOutput=False)
    wk = nc.declare_dram_parameter("wk", [HD, H], F16, isOutput=False)
    wv8 = nc.declare_dram_parameter("wv8", [HD, H], I8, isOutput=False)
    wo8 = nc.declare_dram_parameter("wo8", [EL, H], I8, isOutput=False)
    wsc = nc.declare_dram_parameter("wsc", [128, 3 * KT], F32, isOutput=False)
    rotT = nc.declare_dram_parameter("rotT", [HD, HD], F16, isOutput=False)
    if with_lora:
        laT = nc.declare_dram_parameter("laT", [H, 3 * LR], F16, isOutput=False)
        qbT = nc.declare_dram_parameter("qbT", [LR, EL], F16, isOutput=False)
        kbT = nc.declare_dram_parameter("kbT", [LR, HD], F16, isOutput=False)
        vbT = nc.declare_dram_parameter("vbT", [LR, HD], F16, isOutput=False)
        oaT = nc.declare_dram_parameter("oaT", [H, LR], F16, isOutput=False)
        obT = nc.declare_dram_parameter("obT", [LR, EL], F16, isOutput=False)
    if not causal_ok:
        maskT = nc.declare_dram_parameter("maskT", [S, S], F16, isOutput=False)
    oT_out = nc.declare_dram_parameter("oT_out", [EL, S], F16, isOutput=True)

    with TileContext(nc) as tc:
        with (
            tc.tile_pool(name="const", bufs=1) as const,
            tc.tile_pool(name="persist", bufs=1) as persist,
            tc.tile_pool(name="dram", bufs=1, space="DRAM") as dram,
        ):
            ident = const.tile([128, 128], F32)
            make_identity(nc, ident)
            ones = const.tile([128, 1], F16)
            nc.vector.memset(ones, 1.0)
            rt_sb = const.tile([HD, HD], F16)
            nc.sync.dma_start(out=rt_sb, in_=rotT[:, :])
            if causal_ok:
                # diagonal-block masks depend only on the relative k-tile j:
                # allowed iff 128*j + dk <= dq
                mq = const.tile([128, 4, 512], F32)
                for j in range(4):
                    nc.gpsimd.memset(mq[:, j, :], 0.0)
                    nc.gpsimd.affine_select(
                        out=mq[:, j, :], in_=mq[:, j, :],
                        compare_op=ALU.is_ge, fill=-1e9,
                        base=-128 * j, pattern=[[1, 512]],
                        channel_multiplier=-1)
            if with_lora:
                qb_sb = const.tile([LR, EL], F16)
                nc.sync.dma_start(out=qb_sb, in_=qbT[:, :])
                kb_sb = const.tile([LR, HD], F16)
                nc.sync.dma_start(out=kb_sb, in_=kbT[:, :])
                vb_sb = const.tile([LR, HD], F16)
                nc.sync.dma_start(out=vb_sb, in_=vbT[:, :])
                ob_sb = const.tile([LR, EL], F16)
                nc.sync.dma_start(out=ob_sb, in_=obT[:, :])

            qT_sb = persist.tile([128, QH * S], F16)     # head hh at cols hh*S
            kT_sb = persist.tile([128, S], F16)
            v_sd = persist.tile([128, NST * 128], F16)   # V[s,d], s-tile t at cols t*128
            wo_sb = persist.tile([128, KT, EL], F16)     # lives until stage 3
            if with_lora:
                oa_sb = persist.tile([128, KT, LR], F16)
                nc.sync.dma_start(
                    out=oa_sb, in_=oaT.rearrange("(k p) m -> p k m", p=128))

            # x AllGather: each rank ships rows [c*SSH, (c+1)*SSH) of the
            # packed [S, XA] (x | cos | sin) tensor; AG rebuilds it in order.
            xg_in = dram.tile([SSH, XA], F16, name="xg_in", tag="xg_in")
            xg = dram.tile([S, XA], F16, name="xg", tag="xg",
                           addr_space="Shared" if with_collective else "Local")
            nc.sync.dma_start(out=xg_in[:, :], in_=x_sh[:, :])
            if with_collective:
                nc.gpsimd.collective_compute(
                    "AllGather", ALU.bypass,
                    replica_groups=[list(range(NCORES))],
                    ins=[xg_in[:, :]], outs=[xg[:, :]])
            else:
                nc.sync.dma_start(out=xg[0:SSH, :], in_=xg_in[:, :])

            ag_in = [dram.tile([EL, 512], F16, name=f"ag_in{i}", tag=f"ag_in{i}")
                     for i in range(NSC)]
            ag_out = [dram.tile(
                [NCORES * EL, 512], F16, name=f"ag_out{i}", tag=f"ag_out{i}",
                addr_space="Shared" if with_collective else "Local")
                for i in range(NSC)]

            # ---------------- stage 1: q/k/v (+lora) projections ----------
            with (
                tc.tile_pool(name="s1w", bufs=1) as s1w,
                tc.tile_pool(name="s1x", bufs=6) as s1x,
                tc.tile_pool(name="s1t", bufs=2) as s1t,
                tc.tile_pool(name="s1tab", bufs=1) as s1tab,
                tc.tile_pool(name="s1p", bufs=1, space="PSUM") as s1p,
                tc.tile_pool(name="s1pv", bufs=1, space="PSUM") as s1pv,
            ):
                wq_sb = s1w.tile([128, KT, EL], F16)
                wk_sb = s1w.tile([128, KT, HD], F16)
                wv_sb = s1w.tile([128, KT, HD], F16)
                # k: fp16 XBAR transpose-load; q/v/o: int8 natural-layout
                # staging -> fp32 -> PE transpose -> scale-dequant fp16
                for kt in range(KT):
                    nc.sync.dma_start_transpose(
                        out=wk_sb[:, kt, :], in_=wk[:, kt * 128:(kt + 1) * 128])
                wsc_sb = s1w.tile([128, 3 * KT], F32)
                nc.sync.dma_start(out=wsc_sb, in_=wsc[:, :])
                qvo8 = [
                    (s1w.tile([128, 4, H], I8, name="wq8_sb", tag="stg8"),
                     wq8, wq_sb, 0, 4),
                    (s1w.tile([128, 1, H], I8, name="wv8_sb", tag="stg8v"),
                     wv8, wv_sb, 1, 1),
                    (s1w.tile([128, 4, H], I8, name="wo8_sb", tag="stg8"),
                     wo8, wo_sb, 2, 4),
                ]
                for sb8, src, dst, wi, nb in qvo8:
                    nc.sync.dma_start(
                        out=sb8, in_=src.rearrange("(b p) h -> p b h", p=128))
                    for b in range(nb):
                        stg32 = s1w.tile([128, H], F32, name=f"stg32_{wi}_{b}",
                                         tag="stg32")
                        nc.vector.tensor_copy(stg32, sb8[:, b, :])
                        for kt in range(KT):
                            ptr = s1pv.tile([128, 512], F32, tag="aux",
                                            name=f"wt_{wi}_{b}_{kt}")[:, 0:128]
                            nc.tensor.transpose(
                                ptr, stg32[:, kt * 128:(kt + 1) * 128], ident)
                            nc.scalar.activation(
                                dst[:, kt, b * 128:(b + 1) * 128], ptr, AF.Copy,
                                scale=wsc_sb[:, wi * KT + kt: wi * KT + kt + 1])
                if with_lora:
                    la_sb = s1w.tile([128, KT, 3 * LR], F16)
                    nc.sync.dma_start(
                        out=la_sb,
                        in_=laT.rearrange("(k p) m -> p k m", p=128))

                for sc in range(NSC):
                    ssl = slice(sc * 512, (sc + 1) * 512)
                    pq = [s1p.tile([128, 512], F32, tag=f"pq{et}", name=f"pq{et}_{sc}")
                          for et in range(QH)]
                    pk = s1p.tile([128, 512], F32, tag="pk", name=f"pk_{sc}")
                    pv = s1p.tile([128, 512], F32, tag="pv", name=f"pv_{sc}")
                    pla = (s1p.tile([3 * LR, 512], F32, tag="pla",
                                    name=f"pla_{sc}") if with_lora else None)
                    for kt in range(KT):
                        x_sb = s1x.tile([128, 512], F16, name=f"x_{sc}_{kt}", tag="x")
                        nc.sync.dma_start_transpose(
                            out=x_sb, in_=xg[ssl, kt * 128:(kt + 1) * 128])
                        st = (kt == 0)
                        for et in range(QH):
                            nc.tensor.matmul(pq[et], wq_sb[:, kt, et * 128:(et + 1) * 128],
                                             x_sb, start=st,
                                             stop=(kt == KT - 1) and not with_lora)
                        lastk = (kt == KT - 1)
                        nc.tensor.matmul(pk, wk_sb[:, kt, :], x_sb, start=st,
                                         stop=lastk and not with_lora)
                        nc.tensor.matmul(pv, wv_sb[:, kt, :], x_sb, start=st,
                                         stop=lastk and not with_lora)
                        if with_lora:
                            nc.tensor.matmul(pla, la_sb[:, kt, :], x_sb, start=st,
                                             stop=lastk)
                    if with_lora:
                        laq = s1t.tile([3 * LR, 512], F16, name=f"laq_{sc}", tag="laq")
                        nc.vector.tensor_copy(laq, pla)
                        lak = s1t.tile([LR, 512], F16, name=f"lak_{sc}", tag="lak")
                        nc.sync.dma_start(out=lak, in_=laq[LR:2 * LR, :])
                        lav = s1t.tile([LR, 512], F16, name=f"lav_{sc}", tag="lav")
                        nc.sync.dma_start(out=lav, in_=laq[2 * LR:3 * LR, :])
                        for et in range(QH):
                            nc.tensor.matmul(pq[et], qb_sb[:, et * 128:(et + 1) * 128],
                                             laq[0:LR, :], start=False, stop=True)
                        nc.tensor.matmul(pk, kb_sb, lak, start=False, stop=True)
                        nc.tensor.matmul(pv, vb_sb, lav, start=False, stop=True)

                    # rope tables for this chunk (packed behind x in xg)
                    cq = s1tab.tile([HD, 512], F16, name=f"cq_{sc}", tag="cq")
                    nc.sync.dma_start_transpose(out=cq, in_=xg[ssl, H:H + HD])
                    sq = s1tab.tile([HD, 512], F16, name=f"sq_{sc}", tag="sq")
                    nc.sync.dma_start_transpose(out=sq, in_=xg[ssl, H + HD:H + 2 * HD])

                    # rope: out = p*cos + (R @ p)*sin
                    for et in range(QH + 1):
                        src = pq[et] if et < QH else pk
                        raw = s1t.tile([128, 512], F16, name=f"raw_{sc}_{et}", tag="raw")
                        nc.vector.tensor_copy(raw, src)
                        prot = s1pv.tile([128, 512], F32, tag="aux",
                                         name=f"prot_{sc}_{et}")
                        nc.tensor.matmul(prot, rt_sb, raw, start=True, stop=True)
                        t1 = s1t.tile([128, 512], F32, name=f"t1_{sc}_{et}", tag="t1")
                        nc.vector.tensor_tensor(out=t1, in0=src, in1=cq, op=ALU.mult)
                        t2 = s1t.tile([128, 512], F32, name=f"t2_{sc}_{et}", tag="t2")
                        nc.vector.tensor_tensor(out=t2, in0=prot, in1=sq, op=ALU.mult)
                        if et < QH:
                            dst = qT_sb[:, et * S + sc * 512: et * S + (sc + 1) * 512]
                        else:
                            dst = kT_sb[:, ssl]
                        nc.vector.tensor_tensor(out=dst, in0=t1, in1=t2, op=ALU.add)

                    # v: transpose [d,s]->[s,d] tiles
                    v_sb = s1t.tile([128, 512], F32, name=f"vsb_{sc}", tag="vsb")
                    nc.vector.tensor_copy(v_sb, pv)
                    for j in range(4):
                        stt = 4 * sc + j
                        pvt = s1pv.tile([128, 512], F32, tag="aux",
                                        name=f"pvt_{sc}_{j}")[:, 0:128]
                        nc.tensor.transpose(pvt, v_sb[:, j * 128:(j + 1) * 128], ident)
                        nc.vector.tensor_copy(v_sd[:, stt * 128:(stt + 1) * 128], pvt)

            # ------------- stage 2: attention + stage 3: o projection ------
            with (
                tc.tile_pool(name="s2m", bufs=2) as s2m,
                tc.tile_pool(name="s2t", bufs=4) as s2t,
                tc.tile_pool(name="s3a", bufs=8) as s3a,
                tc.tile_pool(name="s3t", bufs=2) as s3t,
            ):
                s2psum = tc.tile_pool(name="s2ps", bufs=3, space="PSUM")
                s2ps = s2psum.__enter__()
                s2posum = tc.tile_pool(name="s2po", bufs=2, space="PSUM")
                s2po = s2posum.__enter__()
                for qc in range(NSC):
                    qsl = slice(qc * 512, (qc + 1) * 512)
                    if not causal_ok:
                        mqf = s2m.tile([128, NST, 512], F16, name=f"mq_{qc}",
                                       tag="mq")
                        nc.sync.dma_start(
                            out=mqf,
                            in_=maskT.rearrange("(t p) q -> p t q", p=128)[:, :, qsl])
                    nkt = 4 * qc + 4 if causal_ok else NST
                    for hh in range(QH):
                        p_o = s2po.tile([128, 512], F32, tag="p_o",
                                        name=f"po_{qc}_{hh}")
                        p_den = s2po.tile([1, 512], F32, tag="p_den",
                                          name=f"pden_{qc}_{hh}")
                        for kt in range(nkt):
                            p_s = s2ps.tile([128, 512], F32, tag="p_s",
                                            name=f"psc_{qc}_{hh}_{kt}")
                            nc.tensor.matmul(p_s, kT_sb[:, kt * 128:(kt + 1) * 128],
                                             qT_sb[:, hh * S + qc * 512:
                                                   hh * S + (qc + 1) * 512],
                                             start=True, stop=True)
                            pt = s2t.tile([128, 512], F16,
                                          name=f"pt_{qc}_{hh}_{kt}", tag="pt")
                            di = kt - 4 * qc if causal_ok else kt
                            if causal_ok and 0 <= di < 4:
                                sm = s2t.tile([128, 512], F32,
                                              name=f"sm_{qc}_{hh}_{kt}", tag="sm")
                                nc.vector.tensor_tensor(out=sm, in0=p_s,
                                                        in1=mq[:, di, :], op=ALU.add)
                                nc.scalar.activation(pt, sm, AF.Exp, scale=SCALE)
                            elif not causal_ok:
                                sm = s2t.tile([128, 512], F32,
                                              name=f"sm_{qc}_{hh}_{kt}", tag="sm")
                                nc.vector.tensor_tensor(out=sm, in0=p_s,
                                                        in1=mqf[:, kt, :], op=ALU.add)
                                nc.scalar.activation(pt, sm, AF.Exp, scale=SCALE)
                            else:
                                nc.scalar.activation(pt, p_s, AF.Exp, scale=SCALE)
                            nc.tensor.matmul(p_o, v_sd[:, kt * 128:(kt + 1) * 128],
                                             pt, start=(kt == 0), stop=(kt == nkt - 1))
                            nc.tensor.matmul(p_den, ones, pt,
                                             start=(kt == 0), stop=(kt == nkt - 1))
                        den_r = s2t.tile([1, 512], F32, name=f"denr_{qc}_{hh}",
                                         tag="den_r")
                        nc.vector.reciprocal(den_r, p_den)
                        den_b = s2t.tile([128, 512], F32, name=f"denb_{qc}_{hh}",
                                         tag="den_b")
                        nc.gpsimd.partition_broadcast(den_b, den_r)
                        ot = s2t.tile([128, 512], F16, name=f"ot_{qc}_{hh}", tag="ot")
                        nc.vector.tensor_tensor(out=ot, in0=p_o, in1=den_b, op=ALU.mult)
                        nc.sync.dma_start(
                            out=ag_in[qc][hh * 128:(hh + 1) * 128, :], in_=ot)

                    if with_collective:
                        nc.gpsimd.collective_compute(
                            "AllGather", ALU.bypass,
                            replica_groups=[list(range(NCORES))],
                            ins=[ag_in[qc][:, :]], outs=[ag_out[qc][:, :]])
                    else:
                        for r in range(NCORES):
                            nc.sync.dma_start(
                                out=ag_out[qc][r * EL:(r + 1) * EL, :],
                                in_=ag_in[qc][:, :])

                s2posum.__exit__(None, None, None)
                s2psum.__exit__(None, None, None)

                s3psum = tc.tile_pool(name="s3p", bufs=1 if with_lora else 2,
                                      space="PSUM")
                s3p = s3psum.__enter__()
                for sc in range(NSC):
                    ssl = slice(sc * 512, (sc + 1) * 512)
                    po3 = [s3p.tile([128, 512], F32, tag=f"po3_{mt}",
                                    name=f"po3_{mt}_{sc}") for mt in range(4)]
                    pto = (s3p.tile([LR, 512], F32, tag="pto", name=f"pto_{sc}")
                           if with_lora else None)
                    for kt in range(KT):
                        a_sb = s3a.tile([128, 512], F16, name=f"a_{sc}_{kt}", tag="a")
                        nc.sync.dma_start(
                            out=a_sb, in_=ag_out[sc][kt * 128:(kt + 1) * 128, :])
                        st = (kt == 0)
                        for mt in range(4):
                            nc.tensor.matmul(po3[mt], wo_sb[:, kt, mt * 128:(mt + 1) * 128],
                                             a_sb, start=st,
                                             stop=(kt == KT - 1) and not with_lora)
                        if with_lora:
                            nc.tensor.matmul(pto, oa_sb[:, kt, :], a_sb, start=st,
                                             stop=(kt == KT - 1))
                    if with_lora:
                        to_sb = s3t.tile([LR, 512], F16, name=f"to_{sc}", tag="to")
                        nc.vector.tensor_copy(to_sb, pto)
                    for mt in range(4):
                        if with_lora:
                            nc.tensor.matmul(po3[mt], ob_sb[:, mt * 128:(mt + 1) * 128],
                                             to_sb, start=False, stop=True)
                        o_sb = s3t.tile([128, 512], F16, name=f"osb_{sc}_{mt}",
                                        tag="osb")
                        nc.vector.tensor_copy(o_sb, po3[mt])
                        nc.sync.dma_start(
                            out=oT_out[mt * 128:(mt + 1) * 128, ssl], in_=o_sb)
                s3psum.__exit__(None, None, None)

    nc.finalize()
    return nc


def _rope_tables(position_ids):
    pos = np.asarray(position_ids[0], dtype=np.float64)            # [S]
    inv = ROPE_THETA ** (-np.arange(0, HD, 2, dtype=np.float64) / HD)  # [64]
    freqs = np.outer(pos, inv)                                     # [S, 64]
    emb = np.concatenate([freqs, freqs], axis=1)                   # [S, HD]
    return np.cos(emb), np.sin(emb)                                # [S, HD] f64


def kernel(hidden_states, attention_mask, position_ids,
           q_w, q_a, q_b, k_w, k_a, k_b, v_w, v_a, v_b, o_w, o_a, o_b):
    global LAST_RUN, _LAST_IN_MAPS
    LAST_RUN = None
    _guard_trace_env()
    gc.collect()  # release prior call's jax executables/buffers promptly
    hidden_states = np.asarray(hidden_states)
    position_ids = np.asarray(position_ids)
    q_w, q_b = np.asarray(q_w), np.asarray(q_b)
    k_w, k_b = np.asarray(k_w), np.asarray(k_b)
    v_w, v_b = np.asarray(v_w), np.asarray(v_b)
    o_w, o_b = np.asarray(o_w), np.asarray(o_b)
    x = hidden_states[0]                                           # [S, H]
    mask = np.asarray(attention_mask[0, 0], dtype=np.float32)      # [q, k]

    # Causal structure check on [q, k]: beyond-diagonal k-blocks fully masked,
    # before-chunk k-blocks additive-0, diagonal 512-blocks exactly causal.
    causal_ok = True
    tri = np.tril(np.ones((512, 512), bool))
    for qc in range(NSC):
        q0, q1 = qc * 512, (qc + 1) * 512
        if mask[q0:q1, q1:].size and not np.all(mask[q0:q1, q1:] <= -1e8):
            causal_ok = False
        if not np.all(mask[q0:q1, :q0] == 0.0):
            causal_ok = False
        blk = mask[q0:q1, q0:q1]
        if not (np.all(blk[tri] == 0.0) and np.all(blk[~tri] <= -1e8)):
            causal_ok = False
        if not causal_ok:
            break

    cos, sin = _rope_tables(position_ids)

    x_aug = np.empty((S, XA), np.float16)
    x_aug[:, :H] = x
    x_aug[:, H:H + HD] = cos
    x_aug[:, H + HD:] = sin

    # int8 quantization for q/v/o: per-core-shard, per-contraction-column
    # scales, round-to-nearest, kept in native [out, in] layout (the device
    # transposes + dequants). k stays fp16 (most error-sensitive path).
    def _quant(w):
        s = np.abs(w).max(axis=0)
        s = np.maximum(s, 1e-30) * (1.0 / 127.0)
        q = np.rint(w * (1.0 / s)[None, :]).astype(np.int8)
        return q, s.astype(np.float32)

    wk16 = k_w.astype(np.float16)

    rotT = np.zeros((HD, HD), np.float16)   # lhsT of rotate_half permutation
    for d in range(64):
        rotT[d + 64, d] = -1.0
        rotT[d, d + 64] = 1.0

    with_lora = not (np.all(q_b == 0) and np.all(k_b == 0)
                     and np.all(v_b == 0) and np.all(o_b == 0))
    if with_lora:
        laT = np.ascontiguousarray(
            np.concatenate([q_a, k_a, v_a], axis=0).T.astype(np.float16))
        oaT = np.ascontiguousarray(o_a.T.astype(np.float16))
    if not causal_ok:
        maskT16 = np.ascontiguousarray(
            np.clip(mask, -60000.0, 60000.0).T.astype(np.float16))

    key = (causal_ok, with_lora)
    if key not in _PROGRAM_CACHE:
        _PROGRAM_CACHE[key] = _build_program(causal_ok, with_lora)
    nc = _PROGRAM_CACHE[key]

    def _core_map(c):
        qsl = slice(c * EL, (c + 1) * EL)
        ksl = slice(c * HD, (c + 1) * HD)
        wq8T, s_q = _quant(q_w[qsl])    # [512, H], [H]
        wv8T, s_v = _quant(v_w[ksl])    # [128, H]
        wo8T, s_o = _quant(o_w[qsl])
        wsc = np.empty((128, 3 * KT), np.float32)
        for wi, s in enumerate((s_q, s_v, s_o)):
            wsc[:, wi * KT:(wi + 1) * KT] = s.reshape(KT, 128).T
        im = {
            "x_sh": x_aug[c * SSH:(c + 1) * SSH],
            "wq8": wq8T,
            "wk": wk16[ksl],
            "wv8": wv8T,
            "wo8": wo8T,
            "wsc": wsc,
            "rotT": rotT,
        }
        if with_lora:
            im.update({
                "laT": laT,
                "qbT": np.ascontiguousarray(
                    (q_b[qsl, :] * LORA_SCALE).T.astype(np.float16)),
                "kbT": np.ascontiguousarray(
                    (k_b[ksl, :] * LORA_SCALE).T.astype(np.float16)),
                "vbT": np.ascontiguousarray(
                    (v_b[ksl, :] * LORA_SCALE).T.astype(np.float16)),
                "oaT": oaT,
                "obT": np.ascontiguousarray(
                    (o_b[qsl, :] * LORA_SCALE).T.astype(np.float16)),
            })
        if not causal_ok:
            im["maskT"] = maskT16
        return im

    in_maps = [_core_map(c) for c in range(NCORES)]

    _LAST_IN_MAPS = in_maps
    LAST_RUN = run_bass_kernel_spmd(nc, in_maps, core_ids=list(range(NCORES)))
    out = np.empty((B, S, H), np.float32)
    for c in range(NCORES):
        out[0, :, c * EL:(c + 1) * EL] = LAST_RUN.results[c]["oT_out"].T
    return out
